# revision 6
# baseline (speedup 1.0000x reference)
"""GCN (2x GCNConv + BN + ReLU, mean-pool, FC) on 8 TRN2 NeuronCores.

Strategy (1D graph partition by destination node):
- Nodes are permuted into a graph-strided padded space: core c owns graphs
  [8c, 8c+8), each graph padded to a fixed stride S -> per-core slice of
  SLICE = 8*S columns. This makes BN/pool/slicing static across the SPMD
  program (one program, 8 data).
- conv = D^-1/2 (A+I) D^-1/2 (h @ W) is refactored: gather raw table rows
  h[src], scale by dinv[src] inside the indicator matrix, segment-sum via
  PE matmuls into [feat, dst] PSUM windows, scale by dinv[dst] afterwards.
  W1 is applied AFTER aggregation (layer 1 aggregates raw x), W2 BEFORE
  (layer 2 aggregates r1@W2), so only one table exchange is needed.
- Edge phase: dst-sorted edges -> 128-edge chunks -> dma_gather (int16
  indices, lo/hi table halves since idx < 32768) -> per-chunk indicator
  ind[p,j] = (j == dloc[p]) * dinv_src[p] -> matmul accumulation.
- Collectives: one AllGather for the layer-2 table, two tiny AllReduces
  for BN stats. Pooling is core-local; output assembled on host.
"""
import sys

sys.path.insert(0, "/opt/trn_rl_repo")

import numpy as np

import concourse.bass as bass
import concourse.bacc as bacc
import concourse.tile as tile
from concourse import mybir
from concourse.bass_utils import run_bass_kernel_spmd

P = 128
NCORES = 8
NG = 64
GPC = NG // NCORES   # graphs per core
GMAX = 8             # max chunks per dma_gather (1024 idx ucode limit)
BAND = 4             # windows per scheduling band
EPS = 1e-5
F32 = mybir.dt.float32
I16 = mybir.dt.int16
I32 = mybir.dt.int32


# ----------------------------------------------------------------------------
# Host preprocessing: padded node space, per-core edge chunks, static schedule
# ----------------------------------------------------------------------------
def _preprocess(x, edge_index, batch):
    x = np.asarray(x, np.float32)
    edge_index = np.asarray(edge_index, np.int64)
    batch = np.asarray(batch, np.int64)
    n_nodes, n_feat = x.shape

    cnt = np.bincount(batch, minlength=NG).astype(np.int64)          # [64]
    gstart = np.zeros(NG + 1, np.int64)
    gstart[1:] = np.cumsum(cnt)
    S = int(np.ceil(max(int(cnt.max()), 1) / 16.0) * 16)             # stride
    SLICE = GPC * S                                                  # per-core cols
    NWIN = SLICE // P
    HALF = 4 * SLICE
    TOT = 8 * SLICE

    # node -> padded id
    i_in_g = np.arange(n_nodes) - gstart[batch]
    pid = (batch // GPC) * SLICE + (batch % GPC) * S + i_in_g        # [n]

    xpad = np.zeros((TOT, n_feat), np.float32)
    xpad[pid] = x

    src = pid[edge_index[0]]
    dst = pid[edge_index[1]]
    src = np.concatenate([src, pid])
    dst = np.concatenate([dst, pid])                                 # + self loops

    deg = np.bincount(dst, minlength=TOT).astype(np.float64)
    dinv = np.where(deg > 0, 1.0 / np.sqrt(deg), 0.0).astype(np.float32)

    # per-core, per-(window, half) edge counts -> static chunk schedule
    core_of = dst // SLICE
    nch = np.zeros((NWIN, 2), np.int64)
    per_core = []
    for c in range(NCORES):
        m = core_of == c
        s_c, d_c = src[m], dst[m] - c * SLICE
        win = d_c // P
        half = (s_c >= HALF).astype(np.int64)
        key = win * 2 + half
        cnt_wh = np.bincount(key, minlength=NWIN * 2).reshape(NWIN, 2)
        nch = np.maximum(nch, (cnt_wh + P - 1) // P)
        per_core.append((s_c, d_c, key))

    # static gather/chunk schedule (band of windows; lo run then hi run)
    gathers = []     # (half, first_chunk, n_chunks)
    chunks = []      # (window, half)
    for b0 in range(0, NWIN, BAND):
        ws = range(b0, min(b0 + BAND, NWIN))
        for h in (0, 1):
            run = [(w, h) for w in ws for _ in range(int(nch[w, h]))]
            i = 0
            while i < len(run):
                n = min(GMAX, len(run) - i)
                gathers.append((h, len(chunks), n))
                chunks.extend(run[i : i + n])
                i += n
    TC = len(chunks)
    first_chunk = {}
    last_chunk = {}
    for ci, (w, _) in enumerate(chunks):
        first_chunk.setdefault(w, ci)
        last_chunk[w] = ci
    empty_wins = [w for w in range(NWIN) if w not in first_chunk]

    slots_of = {}
    for ci, (w, h) in enumerate(chunks):
        slots_of.setdefault((w, h), []).append(ci)

    # per-core metadata
    meta = []
    for c in range(NCORES):
        s_c, d_c, key = per_core[c]
        gidx_c = np.zeros((TC, P), np.int64)
        dloc_c = np.zeros((TC, P), np.float32)
        dsrc_c = np.zeros((TC, P), np.float32)
        order = np.argsort(key, kind="stable")
        so, do = s_c[order], d_c[order]
        ko = key[order]
        bounds = np.searchsorted(ko, np.arange(NWIN * 2 + 1))
        for w in range(NWIN):
            for h in (0, 1):
                k = w * 2 + h
                lo_i, hi_i = int(bounds[k]), int(bounds[k + 1])
                ne = hi_i - lo_i
                if ne == 0:
                    continue
                cix = slots_of[(w, h)]
                flat_g = np.zeros(len(cix) * P, np.int64)
                flat_l = np.zeros(len(cix) * P, np.float32)
                flat_s = np.zeros(len(cix) * P, np.float32)
                ss = so[lo_i:hi_i]
                flat_g[:ne] = ss - (HALF if h else 0)
                flat_l[:ne] = (do[lo_i:hi_i] % P).astype(np.float32)
                flat_s[:ne] = dinv[ss]
                for j, ci in enumerate(cix):
                    gidx_c[ci] = flat_g[j * P : (j + 1) * P]
                    dloc_c[ci] = flat_l[j * P : (j + 1) * P]
                    dsrc_c[ci] = flat_s[j * P : (j + 1) * P]
        # wrap indices per gather: [16, n*8] tiled to 128 partitions
        gidx_w = np.zeros((P, TC * 8), np.int16)
        for h, c0, n in gathers:
            blk = gidx_c[c0 : c0 + n].reshape(n * P)
            w16 = blk.reshape(-1, 16).T.astype(np.int16)
            gidx_w[:, c0 * 8 : (c0 + n) * 8] = np.tile(w16, (8, 1))
        cnt_core = cnt[c * GPC : (c + 1) * GPC].astype(np.float32)
        meta.append(
            dict(
                gidx=np.ascontiguousarray(gidx_w),
                dloc=np.ascontiguousarray(dloc_c.T.astype(np.float32)),
                dsrc=np.ascontiguousarray(dsrc_c.T.astype(np.float32)),
                dinvrep=np.ascontiguousarray(
                    np.tile(dinv[c * SLICE : (c + 1) * SLICE][None, :], (P, 1))
                ),
                padcnt=np.ascontiguousarray(
                    np.tile((S - cnt_core)[None, :], (P, 1)).astype(np.float32)
                ),
                cntg=np.maximum(cnt_core, 1.0).reshape(GPC, 1).astype(np.float32),
            )
        )

    sched = dict(
        S=S, SLICE=SLICE, NWIN=NWIN, HALF=HALF, TOT=TOT, TC=TC,
        gathers=gathers, chunks=chunks, first=first_chunk, last=last_chunk,
        empty_wins=empty_wins, n_nodes=n_nodes, n_feat=n_feat,
    )
    return sched, xpad, meta


# ----------------------------------------------------------------------------
# Device program
# ----------------------------------------------------------------------------
def _edge_phase(nc, sc, sb, ps, gidx_s, dloc_s, dsrc_s, iota_f, tab_lo, tab_hi,
                dinvrep_s, yagg, tag):
    """Gather + indicator + segment matmuls; writes yagg = dinv_dst * agg."""
    psums = {}
    for h, c0, n in sc["gathers"]:
        gat = sb.tile([P, n * P], F32, tag="gat", bufs=2,
                      name=f"gat{tag}_{c0}")
        nc.gpsimd.dma_gather(
            out_ap=gat[:, : n * P].rearrange("p (c d) -> p c d", d=P),
            in_ap=tab_hi if h else tab_lo,
            idxs_ap=gidx_s[:, c0 * 8 : (c0 + n) * 8],
            num_idxs=n * P,
            num_idxs_reg=n * P,
            elem_size=P,
        )
        for j in range(n):
            ci = c0 + j
            w = sc["chunks"][ci][0]
            ind = sb.tile([P, P], F32, tag="ind", bufs=4, name=f"ind{tag}_{ci}")
            nc.vector.tensor_scalar(
                out=ind[:],
                in0=iota_f[:],
                scalar1=dloc_s[:, ci : ci + 1],
                scalar2=dsrc_s[:, ci : ci + 1],
                op0=mybir.AluOpType.is_equal,
                op1=mybir.AluOpType.mult,
            )
            if w not in psums:
                psums[w] = ps.tile([P, P], F32, space="PSUM", tag="acc",
                                   bufs=BAND + 1, name=f"acc{tag}_{w}")
            nc.tensor.matmul(
                out=psums[w][:],
                lhsT=gat[:, j * P : (j + 1) * P],
                rhs=ind[:],
                start=(ci == sc["first"][w]),
                stop=(ci == sc["last"][w]),
            )
            if ci == sc["last"][w]:
                nc.vector.tensor_tensor(
                    out=yagg[:, w * P : (w + 1) * P],
                    in0=psums[w][:],
                    in1=dinvrep_s[:, w * P : (w + 1) * P],
                    op=mybir.AluOpType.mult,
                )
                del psums[w]
    for w in sc["empty_wins"]:
        nc.vector.memset(yagg[:, w * P : (w + 1) * P], 0.0)


def _stats(nc, sb, src_tile, scratch, ncols, tag):
    """[P,2] tile with (sum, sum_sq) over free dim; scratch same size."""
    st = sb.tile([P, 2], F32, name=f"st{tag}")
    nc.vector.reduce_sum(st[:, 0:1], src_tile[:, :ncols], axis=mybir.AxisListType.X)
    nc.vector.tensor_tensor(out=scratch[:, :ncols], in0=src_tile[:, :ncols],
                            in1=src_tile[:, :ncols], op=mybir.AluOpType.mult)
    nc.vector.reduce_sum(st[:, 1:2], scratch[:, :ncols], axis=mybir.AxisListType.X)
    return st


def _bn_affine(nc, sb, dr, st, gamma_s, beta_s, inv_n, tag):
    """AllReduce stats; return (scale, bias) [P,1] tiles for relu(s*y+b)."""
    ar_in = dr.tile([P, 2], F32, name=f"arin{tag}")
    ar_out = dr.tile([P, 2], F32, addr_space="Shared", name=f"arout{tag}")
    nc.gpsimd.dma_start(ar_in[:], st[:])
    nc.gpsimd.collective_compute(
        "AllReduce", mybir.AluOpType.add,
        replica_groups=[list(range(NCORES))],
        ins=[ar_in.opt()], outs=[ar_out.opt()],
    )
    g = sb.tile([P, 2], F32, name=f"g{tag}")
    nc.sync.dma_start(g[:], ar_out[:])
    mom = sb.tile([P, 2], F32, name=f"mom{tag}")
    nc.vector.tensor_scalar_mul(mom[:], g[:], inv_n)
    var = sb.tile([P, 1], F32, name=f"var{tag}")
    nc.vector.tensor_tensor(out=var[:], in0=mom[:, 0:1], in1=mom[:, 0:1],
                            op=mybir.AluOpType.mult)
    nc.vector.tensor_tensor(out=var[:], in0=mom[:, 1:2], in1=var[:],
                            op=mybir.AluOpType.subtract)
    eps_t = sb.tile([P, 1], F32, name=f"eps{tag}")
    nc.vector.memset(eps_t[:], float(EPS))
    sd = sb.tile([P, 1], F32, name=f"sd{tag}")
    nc.scalar.activation(sd[:], var[:], mybir.ActivationFunctionType.Sqrt,
                         bias=eps_t[:, 0:1], scale=1.0)
    inv = sb.tile([P, 1], F32, name=f"inv{tag}")
    nc.vector.reciprocal(inv[:], sd[:])
    scl = sb.tile([P, 1], F32, name=f"scl{tag}")
    nc.vector.tensor_tensor(out=scl[:], in0=gamma_s[:], in1=inv[:],
                            op=mybir.AluOpType.mult)
    bia = sb.tile([P, 1], F32, name=f"bia{tag}")
    nc.vector.tensor_tensor(out=bia[:], in0=mom[:, 0:1], in1=scl[:],
                            op=mybir.AluOpType.mult)
    nc.vector.tensor_tensor(out=bia[:], in0=beta_s[:], in1=bia[:],
                            op=mybir.AluOpType.subtract)
    return scl, bia


def _build_program(sc):
    SLICE, NWIN, HALF, TOT, TC = (sc["SLICE"], sc["NWIN"], sc["HALF"],
                                  sc["TOT"], sc["TC"])
    n_feat = sc["n_feat"]
    nc = bacc.Bacc("TRN2", target_bir_lowering=False, debug=False,
                   num_devices=NCORES)

    xlo_d = nc.dram_tensor("xlo", [HALF, n_feat], F32, kind="ExternalInput")
    xhi_d = nc.dram_tensor("xhi", [HALF, n_feat], F32, kind="ExternalInput")
    gidx_d = nc.dram_tensor("gidx", [P, TC * 8], I16, kind="ExternalInput")
    dloc_d = nc.dram_tensor("dloc", [P, TC], F32, kind="ExternalInput")
    dsrc_d = nc.dram_tensor("dsrc", [P, TC], F32, kind="ExternalInput")
    dinvrep_d = nc.dram_tensor("dinvrep", [P, SLICE], F32, kind="ExternalInput")
    padcnt_d = nc.dram_tensor("padcnt", [P, GPC], F32, kind="ExternalInput")
    cntg_d = nc.dram_tensor("cntg", [GPC, 1], F32, kind="ExternalInput")
    w1_d = nc.dram_tensor("w1", [n_feat, P], F32, kind="ExternalInput")
    w2_d = nc.dram_tensor("w2", [P, P], F32, kind="ExternalInput")
    wfc_d = nc.dram_tensor("wfc", [P, 10], F32, kind="ExternalInput")
    gb_d = nc.dram_tensor("gb", [P, 4], F32, kind="ExternalInput")
    bfc_d = nc.dram_tensor("bfc", [GPC, 10], F32, kind="ExternalInput")
    yout_d = nc.dram_tensor("yout", [GPC, 10], F32, kind="ExternalOutput")

    with tile.TileContext(nc) as tc:
        with tc.tile_pool(name="sbuf", bufs=1) as sb, \
             tc.tile_pool(name="psum", bufs=1, space="PSUM") as ps, \
             tc.tile_pool(name="dram", bufs=1, space="DRAM") as dr:

            iota_i = sb.tile([P, P], I32)
            nc.gpsimd.iota(iota_i[:], pattern=[[1, P]], base=0,
                           channel_multiplier=0)
            iota_f = sb.tile([P, P], F32)
            nc.vector.tensor_copy(iota_f[:], iota_i[:])

            gidx_s = sb.tile([P, TC * 8], I16)
            nc.sync.dma_start(gidx_s[:], gidx_d[:, :])
            dloc_s = sb.tile([P, TC], F32)
            nc.sync.dma_start(dloc_s[:], dloc_d[:, :])
            dsrc_s = sb.tile([P, TC], F32)
            nc.sync.dma_start(dsrc_s[:], dsrc_d[:, :])
            dinvrep_s = sb.tile([P, SLICE], F32)
            nc.sync.dma_start(dinvrep_s[:], dinvrep_d[:, :])
            w1_s = sb.tile([n_feat, P], F32)
            nc.sync.dma_start(w1_s[:], w1_d[:, :])
            w2_s = sb.tile([P, P], F32)
            nc.sync.dma_start(w2_s[:], w2_d[:, :])
            gb_s = sb.tile([P, 4], F32)
            nc.sync.dma_start(gb_s[:], gb_d[:, :])

            # three big shared buffers (see reuse plan in comments below)
            bufA = sb.tile([P, SLICE], F32)   # yagg1 -> sq1 scratch -> y2r
            bufB = sb.tile([P, SLICE], F32)   # y1c  -> yagg2
            bufC = sb.tile([P, SLICE], F32)   # r1   -> sq2 scratch

            # ---------------- layer 1: aggregate raw x ----------------
            _edge_phase(nc, sc, sb, ps, gidx_s, dloc_s, dsrc_s, iota_f,
                        xlo_d[:, :], xhi_d[:, :], dinvrep_s, bufA, tag="a")

            # GEMM1: y1c = W1.T @ yagg  (feat-major)
            for w in range(NWIN):
                pg = ps.tile([P, P], F32, space="PSUM", tag="gemm", bufs=2,
                             name=f"g1_{w}")
                nc.tensor.matmul(out=pg[:], lhsT=w1_s[:],
                                 rhs=bufA[:, w * P : (w + 1) * P],
                                 start=True, stop=True)
                nc.vector.tensor_copy(bufB[:, w * P : (w + 1) * P], pg[:])

            st1 = _stats(nc, sb, bufB, bufA, SLICE, "1")
            s1, b1 = _bn_affine(nc, sb, dr, st1, gb_s[:, 0:1], gb_s[:, 1:2],
                                1.0 / sc["n_nodes"], "1")
            # r1 = relu(s1*y1c + b1)
            nc.scalar.activation(bufC[:], bufB[:],
                                 mybir.ActivationFunctionType.Relu,
                                 bias=b1[:, 0:1], scale=s1[:, 0:1])

            # GEMM2: table2 slice = r1.T @ W2 (node-major rows)
            ag_in = dr.tile([SLICE, P], F32)
            for w in range(NWIN):
                pg2 = ps.tile([P, P], F32, space="PSUM", tag="gemm", bufs=2,
                              name=f"g2_{w}")
                nc.tensor.matmul(out=pg2[:], lhsT=bufC[:, w * P : (w + 1) * P],
                                 rhs=w2_s[:], start=True, stop=True)
                t2w = sb.tile([P, P], F32, tag="t2w", bufs=3, name=f"t2w_{w}")
                nc.vector.tensor_copy(t2w[:], pg2[:])
                nc.sync.dma_start(ag_in[w * P : (w + 1) * P, :], t2w[:])

            t2_full = dr.tile([TOT, P], F32, addr_space="Shared")
            nc.gpsimd.collective_compute(
                "AllGather", mybir.AluOpType.bypass,
                replica_groups=[list(range(NCORES))],
                ins=[ag_in.opt()], outs=[t2_full.opt()],
            )

            # ---------------- layer 2: aggregate table2 ----------------
            _edge_phase(nc, sc, sb, ps, gidx_s, dloc_s, dsrc_s, iota_f,
                        t2_full[0:HALF, :], t2_full[HALF:TOT, :],
                        dinvrep_s, bufB, tag="b")

            st2 = _stats(nc, sb, bufB, bufC, SLICE, "2")
            s2, b2 = _bn_affine(nc, sb, dr, st2, gb_s[:, 2:3], gb_s[:, 3:4],
                                1.0 / sc["n_nodes"], "2")
            # y2r = relu(s2*y2 + b2)
            nc.scalar.activation(bufA[:], bufB[:],
                                 mybir.ActivationFunctionType.Relu,
                                 bias=b2[:, 0:1], scale=s2[:, 0:1])

            # pool: per-graph sums minus pad correction
            pooled = sb.tile([P, GPC], F32)
            for j in range(GPC):
                nc.vector.reduce_sum(pooled[:, j : j + 1],
                                     bufA[:, j * sc["S"] : (j + 1) * sc["S"]],
                                     axis=mybir.AxisListType.X)
            rb = sb.tile([P, 1], F32)
            nc.scalar.activation(rb[:], b2[:, 0:1],
                                 mybir.ActivationFunctionType.Relu)
            padcnt_s = sb.tile([P, GPC], F32)
            nc.sync.dma_start(padcnt_s[:], padcnt_d[:, :])
            corr = sb.tile([P, GPC], F32)
            nc.vector.tensor_scalar_mul(corr[:], padcnt_s[:], rb[:, 0:1])
            nc.vector.tensor_tensor(out=pooled[:], in0=pooled[:], in1=corr[:],
                                    op=mybir.AluOpType.subtract)

            # FC: out[g, cls] = pooled.T @ Wfc / cnt + bfc
            wfc_s = sb.tile([P, 10], F32)
            nc.sync.dma_start(wfc_s[:], wfc_d[:, :])
            cnt_s = sb.tile([GPC, 1], F32)
            nc.sync.dma_start(cnt_s[:], cntg_d[:, :])
            cinv = sb.tile([GPC, 1], F32)
            nc.vector.reciprocal(cinv[:], cnt_s[:])
            bfc_s = sb.tile([GPC, 10], F32)
            nc.sync.dma_start(bfc_s[:], bfc_d[:, :])
            pfc = ps.tile([GPC, 10], F32, space="PSUM")
            nc.tensor.matmul(out=pfc[:], lhsT=pooled[:], rhs=wfc_s[:],
                             start=True, stop=True)
            yo = sb.tile([GPC, 10], F32)
            nc.vector.tensor_scalar_mul(yo[:], pfc[:], cinv[:, 0:1])
            nc.vector.tensor_tensor(out=yo[:], in0=yo[:], in1=bfc_s[:],
                                    op=mybir.AluOpType.add)
            nc.sync.dma_start(yout_d[:, :], yo[:])

    nc.compile()
    return nc


# ----------------------------------------------------------------------------
# Entry point
# ----------------------------------------------------------------------------
def _make_in_maps(sc, xpad, meta, inputs):
    HALF = sc["HALF"]
    gb = np.stack([np.asarray(inputs["gamma1"], np.float32),
                   np.asarray(inputs["beta1"], np.float32),
                   np.asarray(inputs["gamma2"], np.float32),
                   np.asarray(inputs["beta2"], np.float32)], axis=1)  # [128, 4]
    common = dict(
        xlo=np.ascontiguousarray(xpad[:HALF]),
        xhi=np.ascontiguousarray(xpad[HALF:]),
        w1=np.ascontiguousarray(np.asarray(inputs["W1"], np.float32)),
        w2=np.ascontiguousarray(np.asarray(inputs["W2"], np.float32)),
        wfc=np.ascontiguousarray(np.asarray(inputs["Wfc"], np.float32)),
        gb=np.ascontiguousarray(gb),
    )
    bfc_rep = np.ascontiguousarray(
        np.tile(np.asarray(inputs["bfc"], np.float32)[None, :], (GPC, 1)))
    in_maps = []
    for c in range(NCORES):
        m = meta[c]
        in_maps.append(dict(common, gidx=m["gidx"], dloc=m["dloc"],
                            dsrc=m["dsrc"], dinvrep=m["dinvrep"],
                            padcnt=m["padcnt"], cntg=m["cntg"], bfc=bfc_rep))
    return in_maps


def kernel(x, edge_index, batch, W1, b1, gamma1, beta1, W2, b2, gamma2, beta2,
           Wfc, bfc, _trace=False):
    sc, xpad, meta = _preprocess(x, edge_index, batch)
    nc = _build_program(sc)
    in_maps = _make_in_maps(sc, xpad, meta, dict(
        gamma1=gamma1, beta1=beta1, gamma2=gamma2, beta2=beta2,
        W1=W1, W2=W2, Wfc=Wfc, bfc=bfc))

    res = run_bass_kernel_spmd(nc, in_maps, core_ids=list(range(NCORES)),
                               trace=_trace)
    out = np.concatenate([res.results[c]["yout"] for c in range(NCORES)], axis=0)
    if _trace:
        return out.astype(np.float32), res
    return out.astype(np.float32)


# revision 7
# speedup vs baseline: 1.0970x; 1.0970x over previous
"""GCN (2x GCNConv + BN + ReLU, mean-pool, FC) on 8 TRN2 NeuronCores.

Strategy (1D graph partition by destination node):
- Nodes are permuted into a graph-strided padded space: core c owns graphs
  [8c, 8c+8), each graph padded to a fixed stride S -> per-core slice of
  SLICE = 8*S columns. This makes BN/pool/slicing static across the SPMD
  program (one program, 8 data).
- conv = D^-1/2 (A+I) D^-1/2 (h @ W) is refactored: gather raw table rows
  h[src], scale by dinv[src] inside the indicator matrix, segment-sum via
  PE matmuls into [feat, dst] PSUM windows, scale by dinv[dst] afterwards.
  W1 is applied AFTER aggregation (layer 1 aggregates raw x), W2 BEFORE
  (layer 2 aggregates r1@W2), so only one table exchange is needed.
- Edge phase: dst-sorted edges -> 128-edge chunks -> dma_gather (int16
  indices, lo/hi table halves since idx < 32768) -> per-chunk indicator
  ind[p,j] = (j == dloc[p]) * dinv_src[p] -> matmul accumulation.
- Collectives: one AllGather for the layer-2 table, two tiny AllReduces
  for BN stats. Pooling is core-local; output assembled on host.
"""
import sys

sys.path.insert(0, "/opt/trn_rl_repo")

import numpy as np

import concourse.bass as bass
import concourse.bacc as bacc
import concourse.tile as tile
from concourse import mybir
from concourse.bass_utils import run_bass_kernel_spmd

P = 128
NCORES = 8
NG = 64
GPC = NG // NCORES   # graphs per core
GMAX = 8             # max chunks per dma_gather (1024 idx ucode limit)
BAND = 4             # windows per scheduling band
EPS = 1e-5
F32 = mybir.dt.float32
I16 = mybir.dt.int16
I32 = mybir.dt.int32


# ----------------------------------------------------------------------------
# Host preprocessing: padded node space, per-core edge chunks, static schedule
# ----------------------------------------------------------------------------
def _preprocess(x, edge_index, batch):
    x = np.asarray(x, np.float32)
    edge_index = np.asarray(edge_index, np.int64)
    batch = np.asarray(batch, np.int64)
    n_nodes, n_feat = x.shape

    cnt = np.bincount(batch, minlength=NG).astype(np.int64)          # [64]
    gstart = np.zeros(NG + 1, np.int64)
    gstart[1:] = np.cumsum(cnt)
    S = int(np.ceil(max(int(cnt.max()), 1) / 16.0) * 16)             # stride
    SLICE = GPC * S                                                  # per-core cols
    NWIN = SLICE // P
    HALF = 4 * SLICE
    TOT = 8 * SLICE

    # node -> padded id
    i_in_g = np.arange(n_nodes) - gstart[batch]
    pid = (batch // GPC) * SLICE + (batch % GPC) * S + i_in_g        # [n]

    xpad = np.zeros((TOT, n_feat), np.float32)
    xpad[pid] = x

    src = pid[edge_index[0]]
    dst = pid[edge_index[1]]
    src = np.concatenate([src, pid])
    dst = np.concatenate([dst, pid])                                 # + self loops

    deg = np.bincount(dst, minlength=TOT).astype(np.float64)
    dinv = np.where(deg > 0, 1.0 / np.sqrt(deg), 0.0).astype(np.float32)

    # per-core, per-(window, half) edge counts -> static chunk schedule
    core_of = dst // SLICE
    nch = np.zeros((NWIN, 2), np.int64)
    per_core = []
    for c in range(NCORES):
        m = core_of == c
        s_c, d_c = src[m], dst[m] - c * SLICE
        win = d_c // P
        half = (s_c >= HALF).astype(np.int64)
        key = win * 2 + half
        cnt_wh = np.bincount(key, minlength=NWIN * 2).reshape(NWIN, 2)
        nch = np.maximum(nch, (cnt_wh + P - 1) // P)
        per_core.append((s_c, d_c, key))

    # static gather/chunk schedule (band of windows; lo run then hi run)
    gathers = []     # (half, first_chunk, n_chunks)
    chunks = []      # (window, half)
    for b0 in range(0, NWIN, BAND):
        ws = range(b0, min(b0 + BAND, NWIN))
        for h in (0, 1):
            run = [(w, h) for w in ws for _ in range(int(nch[w, h]))]
            i = 0
            while i < len(run):
                n = min(GMAX, len(run) - i)
                gathers.append((h, len(chunks), n))
                chunks.extend(run[i : i + n])
                i += n
    TC = len(chunks)
    first_chunk = {}
    last_chunk = {}
    for ci, (w, _) in enumerate(chunks):
        first_chunk.setdefault(w, ci)
        last_chunk[w] = ci
    empty_wins = [w for w in range(NWIN) if w not in first_chunk]

    slots_of = {}
    for ci, (w, h) in enumerate(chunks):
        slots_of.setdefault((w, h), []).append(ci)

    # per-core metadata
    meta = []
    for c in range(NCORES):
        s_c, d_c, key = per_core[c]
        gidx_c = np.zeros((TC, P), np.int64)
        dloc_c = np.zeros((TC, P), np.float32)
        dsrc_c = np.zeros((TC, P), np.float32)
        order = np.argsort(key, kind="stable")
        so, do = s_c[order], d_c[order]
        ko = key[order]
        bounds = np.searchsorted(ko, np.arange(NWIN * 2 + 1))
        for w in range(NWIN):
            for h in (0, 1):
                k = w * 2 + h
                lo_i, hi_i = int(bounds[k]), int(bounds[k + 1])
                ne = hi_i - lo_i
                if ne == 0:
                    continue
                cix = slots_of[(w, h)]
                flat_g = np.zeros(len(cix) * P, np.int64)
                flat_l = np.zeros(len(cix) * P, np.float32)
                flat_s = np.zeros(len(cix) * P, np.float32)
                ss = so[lo_i:hi_i]
                flat_g[:ne] = ss - (HALF if h else 0)
                flat_l[:ne] = (do[lo_i:hi_i] % P).astype(np.float32)
                flat_s[:ne] = dinv[ss]
                for j, ci in enumerate(cix):
                    gidx_c[ci] = flat_g[j * P : (j + 1) * P]
                    dloc_c[ci] = flat_l[j * P : (j + 1) * P]
                    dsrc_c[ci] = flat_s[j * P : (j + 1) * P]
        # wrap indices per gather: [16, n*8] tiled to 128 partitions
        gidx_w = np.zeros((P, TC * 8), np.int16)
        for h, c0, n in gathers:
            blk = gidx_c[c0 : c0 + n].reshape(n * P)
            w16 = blk.reshape(-1, 16).T.astype(np.int16)
            gidx_w[:, c0 * 8 : (c0 + n) * 8] = np.tile(w16, (8, 1))
        cnt_core = cnt[c * GPC : (c + 1) * GPC].astype(np.float32)
        meta.append(
            dict(
                gidx=np.ascontiguousarray(gidx_w),
                dloc=np.ascontiguousarray(dloc_c.T.astype(np.float32)),
                dsrc=np.ascontiguousarray(dsrc_c.T.astype(np.float32)),
                dinvrep=np.ascontiguousarray(
                    np.tile(dinv[c * SLICE : (c + 1) * SLICE][None, :], (P, 1))
                ),
                padcnt=np.ascontiguousarray(
                    np.tile((S - cnt_core)[None, :], (P, 1)).astype(np.float32)
                ),
                cntg=np.maximum(cnt_core, 1.0).reshape(GPC, 1).astype(np.float32),
            )
        )

    sched = dict(
        S=S, SLICE=SLICE, NWIN=NWIN, HALF=HALF, TOT=TOT, TC=TC,
        gathers=gathers, chunks=chunks, first=first_chunk, last=last_chunk,
        empty_wins=empty_wins, n_nodes=n_nodes, n_feat=n_feat,
    )
    return sched, xpad, meta


# ----------------------------------------------------------------------------
# Device program
# ----------------------------------------------------------------------------
def _edge_phase(nc, sc, sb, ps, gidx_s, dloc_s, dsrc_s, iota_f, tab_lo, tab_hi,
                dinvrep_s, yagg, tag):
    """Gather + indicator + segment matmuls; writes yagg = dinv_dst * agg."""
    psums = {}
    for h, c0, n in sc["gathers"]:
        gat = sb.tile([P, n * P], F32, tag="gat", bufs=2,
                      name=f"gat{tag}_{c0}")
        nc.gpsimd.dma_gather(
            out_ap=gat[:, : n * P].rearrange("p (c d) -> p c d", d=P),
            in_ap=tab_hi if h else tab_lo,
            idxs_ap=gidx_s[:, c0 * 8 : (c0 + n) * 8],
            num_idxs=n * P,
            num_idxs_reg=n * P,
            elem_size=P,
        )
        for j in range(n):
            ci = c0 + j
            w = sc["chunks"][ci][0]
            ind = sb.tile([P, P], F32, tag="ind", bufs=4, name=f"ind{tag}_{ci}")
            nc.vector.tensor_scalar(
                out=ind[:],
                in0=iota_f[:],
                scalar1=dloc_s[:, ci : ci + 1],
                scalar2=dsrc_s[:, ci : ci + 1],
                op0=mybir.AluOpType.is_equal,
                op1=mybir.AluOpType.mult,
            )
            if w not in psums:
                psums[w] = ps.tile([P, P], F32, space="PSUM", tag="acc",
                                   bufs=BAND + 1, name=f"acc{tag}_{w}")
            nc.tensor.matmul(
                out=psums[w][:],
                lhsT=gat[:, j * P : (j + 1) * P],
                rhs=ind[:],
                start=(ci == sc["first"][w]),
                stop=(ci == sc["last"][w]),
            )
            if ci == sc["last"][w]:
                nc.vector.tensor_tensor(
                    out=yagg[:, w * P : (w + 1) * P],
                    in0=psums[w][:],
                    in1=dinvrep_s[:, w * P : (w + 1) * P],
                    op=mybir.AluOpType.mult,
                )
                del psums[w]
    for w in sc["empty_wins"]:
        nc.vector.memset(yagg[:, w * P : (w + 1) * P], 0.0)


def _stats(nc, sb, src_tile, scratch, ncols, tag):
    """[P,2] tile with (sum, sum_sq) over free dim; scratch same size."""
    st = sb.tile([P, 2], F32, name=f"st{tag}")
    nc.vector.reduce_sum(st[:, 0:1], src_tile[:, :ncols], axis=mybir.AxisListType.X)
    nc.vector.tensor_tensor(out=scratch[:, :ncols], in0=src_tile[:, :ncols],
                            in1=src_tile[:, :ncols], op=mybir.AluOpType.mult)
    nc.vector.reduce_sum(st[:, 1:2], scratch[:, :ncols], axis=mybir.AxisListType.X)
    return st


def _bn_affine(nc, sb, dr, st, gamma_s, beta_s, inv_n, tag):
    """AllReduce stats; return (scale, bias) [P,1] tiles for relu(s*y+b)."""
    ar_in = dr.tile([P, 2], F32, name=f"arin{tag}")
    ar_out = dr.tile([P, 2], F32, addr_space="Shared", name=f"arout{tag}")
    nc.gpsimd.dma_start(ar_in[:], st[:])
    nc.gpsimd.collective_compute(
        "AllReduce", mybir.AluOpType.add,
        replica_groups=[list(range(NCORES))],
        ins=[ar_in.opt()], outs=[ar_out.opt()],
    )
    g = sb.tile([P, 2], F32, name=f"g{tag}")
    nc.sync.dma_start(g[:], ar_out[:])
    mom = sb.tile([P, 2], F32, name=f"mom{tag}")
    nc.vector.tensor_scalar_mul(mom[:], g[:], inv_n)
    var = sb.tile([P, 1], F32, name=f"var{tag}")
    nc.vector.tensor_tensor(out=var[:], in0=mom[:, 0:1], in1=mom[:, 0:1],
                            op=mybir.AluOpType.mult)
    nc.vector.tensor_tensor(out=var[:], in0=mom[:, 1:2], in1=var[:],
                            op=mybir.AluOpType.subtract)
    eps_t = sb.tile([P, 1], F32, name=f"eps{tag}")
    nc.vector.memset(eps_t[:], float(EPS))
    sd = sb.tile([P, 1], F32, name=f"sd{tag}")
    nc.scalar.activation(sd[:], var[:], mybir.ActivationFunctionType.Sqrt,
                         bias=eps_t[:, 0:1], scale=1.0)
    inv = sb.tile([P, 1], F32, name=f"inv{tag}")
    nc.vector.reciprocal(inv[:], sd[:])
    scl = sb.tile([P, 1], F32, name=f"scl{tag}")
    nc.vector.tensor_tensor(out=scl[:], in0=gamma_s[:], in1=inv[:],
                            op=mybir.AluOpType.mult)
    bia = sb.tile([P, 1], F32, name=f"bia{tag}")
    nc.vector.tensor_tensor(out=bia[:], in0=mom[:, 0:1], in1=scl[:],
                            op=mybir.AluOpType.mult)
    nc.vector.tensor_tensor(out=bia[:], in0=beta_s[:], in1=bia[:],
                            op=mybir.AluOpType.subtract)
    return scl, bia


def _build_program(sc):
    SLICE, NWIN, HALF, TOT, TC = (sc["SLICE"], sc["NWIN"], sc["HALF"],
                                  sc["TOT"], sc["TC"])
    n_feat = sc["n_feat"]
    nc = bacc.Bacc("TRN2", target_bir_lowering=False, debug=False,
                   num_devices=NCORES)

    xlo_d = nc.dram_tensor("xlo", [HALF, n_feat], F32, kind="ExternalInput")
    xhi_d = nc.dram_tensor("xhi", [HALF, n_feat], F32, kind="ExternalInput")
    gidx_d = nc.dram_tensor("gidx", [P, TC * 8], I16, kind="ExternalInput")
    dloc_d = nc.dram_tensor("dloc", [P, TC], F32, kind="ExternalInput")
    dsrc_d = nc.dram_tensor("dsrc", [P, TC], F32, kind="ExternalInput")
    dinvrep_d = nc.dram_tensor("dinvrep", [P, SLICE], F32, kind="ExternalInput")
    padcnt_d = nc.dram_tensor("padcnt", [P, GPC], F32, kind="ExternalInput")
    cntg_d = nc.dram_tensor("cntg", [GPC, 1], F32, kind="ExternalInput")
    w1_d = nc.dram_tensor("w1", [n_feat, P], F32, kind="ExternalInput")
    w2_d = nc.dram_tensor("w2", [P, P], F32, kind="ExternalInput")
    wfc_d = nc.dram_tensor("wfc", [P, 10], F32, kind="ExternalInput")
    gb_d = nc.dram_tensor("gb", [P, 4], F32, kind="ExternalInput")
    bfc_d = nc.dram_tensor("bfc", [GPC, 10], F32, kind="ExternalInput")
    yout_d = nc.dram_tensor("yout", [GPC, 10], F32, kind="ExternalOutput")

    with tile.TileContext(nc) as tc:
        with tc.tile_pool(name="sbuf", bufs=1) as sb, \
             tc.tile_pool(name="psum", bufs=1, space="PSUM") as ps, \
             tc.tile_pool(name="dram", bufs=1, space="DRAM") as dr:

            iota_i = sb.tile([P, P], I32)
            nc.gpsimd.iota(iota_i[:], pattern=[[1, P]], base=0,
                           channel_multiplier=0)
            iota_f = sb.tile([P, P], F32)
            nc.vector.tensor_copy(iota_f[:], iota_i[:])

            gidx_s = sb.tile([P, TC * 8], I16)
            nc.sync.dma_start(gidx_s[:], gidx_d[:, :])
            dloc_s = sb.tile([P, TC], F32)
            nc.sync.dma_start(dloc_s[:], dloc_d[:, :])
            dsrc_s = sb.tile([P, TC], F32)
            nc.sync.dma_start(dsrc_s[:], dsrc_d[:, :])
            dinvrep_s = sb.tile([P, SLICE], F32)
            nc.sync.dma_start(dinvrep_s[:], dinvrep_d[:, :])
            w1_s = sb.tile([n_feat, P], F32)
            nc.sync.dma_start(w1_s[:], w1_d[:, :])
            w2_s = sb.tile([P, P], F32)
            nc.sync.dma_start(w2_s[:], w2_d[:, :])
            gb_s = sb.tile([P, 4], F32)
            nc.sync.dma_start(gb_s[:], gb_d[:, :])

            # three big shared buffers (see reuse plan in comments below)
            bufA = sb.tile([P, SLICE], F32)   # yagg1 -> sq1 scratch -> y2r
            bufB = sb.tile([P, SLICE], F32)   # y1c  -> yagg2
            bufC = sb.tile([P, SLICE], F32)   # r1   -> sq2 scratch

            # ---------------- layer 1: aggregate raw x ----------------
            _edge_phase(nc, sc, sb, ps, gidx_s, dloc_s, dsrc_s, iota_f,
                        xlo_d[:, :], xhi_d[:, :], dinvrep_s, bufA, tag="a")

            # GEMM1: y1c = W1.T @ yagg  (feat-major)
            for w in range(NWIN):
                pg = ps.tile([P, P], F32, space="PSUM", tag="gemm", bufs=2,
                             name=f"g1_{w}")
                nc.tensor.matmul(out=pg[:], lhsT=w1_s[:],
                                 rhs=bufA[:, w * P : (w + 1) * P],
                                 start=True, stop=True)
                nc.vector.tensor_copy(bufB[:, w * P : (w + 1) * P], pg[:])

            st1 = _stats(nc, sb, bufB, bufA, SLICE, "1")
            s1, b1 = _bn_affine(nc, sb, dr, st1, gb_s[:, 0:1], gb_s[:, 1:2],
                                1.0 / sc["n_nodes"], "1")
            # r1 = relu(s1*y1c + b1)
            nc.scalar.activation(bufC[:], bufB[:],
                                 mybir.ActivationFunctionType.Relu,
                                 bias=b1[:, 0:1], scale=s1[:, 0:1])

            # GEMM2: table2 slice = r1.T @ W2 (node-major rows)
            ag_in = dr.tile([SLICE, P], F32)
            for w in range(NWIN):
                pg2 = ps.tile([P, P], F32, space="PSUM", tag="gemm", bufs=2,
                              name=f"g2_{w}")
                nc.tensor.matmul(out=pg2[:], lhsT=bufC[:, w * P : (w + 1) * P],
                                 rhs=w2_s[:], start=True, stop=True)
                t2w = sb.tile([P, P], F32, tag="t2w", bufs=3, name=f"t2w_{w}")
                nc.vector.tensor_copy(t2w[:], pg2[:])
                nc.sync.dma_start(ag_in[w * P : (w + 1) * P, :], t2w[:])

            t2_full = dr.tile([TOT, P], F32, addr_space="Shared")
            nc.gpsimd.collective_compute(
                "AllGather", mybir.AluOpType.bypass,
                replica_groups=[list(range(NCORES))],
                ins=[ag_in.opt()], outs=[t2_full.opt()],
            )

            # ---------------- layer 2: aggregate table2 ----------------
            _edge_phase(nc, sc, sb, ps, gidx_s, dloc_s, dsrc_s, iota_f,
                        t2_full[0:HALF, :], t2_full[HALF:TOT, :],
                        dinvrep_s, bufB, tag="b")

            st2 = _stats(nc, sb, bufB, bufC, SLICE, "2")
            s2, b2 = _bn_affine(nc, sb, dr, st2, gb_s[:, 2:3], gb_s[:, 3:4],
                                1.0 / sc["n_nodes"], "2")
            # y2r = relu(s2*y2 + b2)
            nc.scalar.activation(bufA[:], bufB[:],
                                 mybir.ActivationFunctionType.Relu,
                                 bias=b2[:, 0:1], scale=s2[:, 0:1])

            # pool: per-graph sums minus pad correction
            pooled = sb.tile([P, GPC], F32)
            for j in range(GPC):
                nc.vector.reduce_sum(pooled[:, j : j + 1],
                                     bufA[:, j * sc["S"] : (j + 1) * sc["S"]],
                                     axis=mybir.AxisListType.X)
            rb = sb.tile([P, 1], F32)
            nc.scalar.activation(rb[:], b2[:, 0:1],
                                 mybir.ActivationFunctionType.Relu)
            padcnt_s = sb.tile([P, GPC], F32)
            nc.sync.dma_start(padcnt_s[:], padcnt_d[:, :])
            corr = sb.tile([P, GPC], F32)
            nc.vector.tensor_scalar_mul(corr[:], padcnt_s[:], rb[:, 0:1])
            nc.vector.tensor_tensor(out=pooled[:], in0=pooled[:], in1=corr[:],
                                    op=mybir.AluOpType.subtract)

            # FC: out[g, cls] = pooled.T @ Wfc / cnt + bfc
            wfc_s = sb.tile([P, 10], F32)
            nc.sync.dma_start(wfc_s[:], wfc_d[:, :])
            cnt_s = sb.tile([GPC, 1], F32)
            nc.sync.dma_start(cnt_s[:], cntg_d[:, :])
            cinv = sb.tile([GPC, 1], F32)
            nc.vector.reciprocal(cinv[:], cnt_s[:])
            bfc_s = sb.tile([GPC, 10], F32)
            nc.sync.dma_start(bfc_s[:], bfc_d[:, :])
            pfc = ps.tile([GPC, 10], F32, space="PSUM")
            nc.tensor.matmul(out=pfc[:], lhsT=pooled[:], rhs=wfc_s[:],
                             start=True, stop=True)
            yo = sb.tile([GPC, 10], F32)
            nc.vector.tensor_scalar_mul(yo[:], pfc[:], cinv[:, 0:1])
            nc.vector.tensor_tensor(out=yo[:], in0=yo[:], in1=bfc_s[:],
                                    op=mybir.AluOpType.add)
            nc.sync.dma_start(yout_d[:, :], yo[:])

    nc.compile()
    return nc


# ----------------------------------------------------------------------------
# Entry point
# ----------------------------------------------------------------------------
def _make_in_maps(sc, xpad, meta, inputs):
    HALF = sc["HALF"]
    gb = np.stack([np.asarray(inputs["gamma1"], np.float32),
                   np.asarray(inputs["beta1"], np.float32),
                   np.asarray(inputs["gamma2"], np.float32),
                   np.asarray(inputs["beta2"], np.float32)], axis=1)  # [128, 4]
    common = dict(
        xlo=np.ascontiguousarray(xpad[:HALF]),
        xhi=np.ascontiguousarray(xpad[HALF:]),
        w1=np.ascontiguousarray(np.asarray(inputs["W1"], np.float32)),
        w2=np.ascontiguousarray(np.asarray(inputs["W2"], np.float32)),
        wfc=np.ascontiguousarray(np.asarray(inputs["Wfc"], np.float32)),
        gb=np.ascontiguousarray(gb),
    )
    bfc_rep = np.ascontiguousarray(
        np.tile(np.asarray(inputs["bfc"], np.float32)[None, :], (GPC, 1)))
    in_maps = []
    for c in range(NCORES):
        m = meta[c]
        in_maps.append(dict(common, gidx=m["gidx"], dloc=m["dloc"],
                            dsrc=m["dsrc"], dinvrep=m["dinvrep"],
                            padcnt=m["padcnt"], cntg=m["cntg"], bfc=bfc_rep))
    return in_maps


def kernel(x, edge_index, batch, W1, b1, gamma1, beta1, W2, b2, gamma2, beta2,
           Wfc, bfc, _trace=False):
    sc, xpad, meta = _preprocess(x, edge_index, batch)
    nc = _build_program(sc)
    in_maps = _make_in_maps(sc, xpad, meta, dict(
        gamma1=gamma1, beta1=beta1, gamma2=gamma2, beta2=beta2,
        W1=W1, W2=W2, Wfc=Wfc, bfc=bfc))

    res = None
    last_err = None
    for _attempt in range(3):
        try:
            res = run_bass_kernel_spmd(nc, in_maps, core_ids=list(range(NCORES)),
                                       trace=_trace)
            break
        except Exception as e:  # transient NRT/axon failures; retry
            last_err = e
    if res is None:
        raise last_err
    out = np.concatenate([res.results[c]["yout"] for c in range(NCORES)], axis=0)
    if _trace:
        return out.astype(np.float32), res
    return out.astype(np.float32)


# revision 8
# speedup vs baseline: 1.1349x; 1.0345x over previous
"""GCN (2x GCNConv + BN + ReLU, mean-pool, FC) on 8 TRN2 NeuronCores.

Strategy (1D graph partition by destination node):
- Nodes are permuted into a graph-strided padded space: core c owns graphs
  [8c, 8c+8), each graph padded to a fixed stride S -> per-core slice of
  SLICE = 8*S columns. This makes BN/pool/slicing static across the SPMD
  program (one program, 8 data).
- conv = D^-1/2 (A+I) D^-1/2 (h @ W) is refactored: gather raw table rows
  h[src], scale by dinv[src] inside the indicator matrix, segment-sum via
  PE matmuls into [feat, dst] PSUM windows, scale by dinv[dst] afterwards.
  W1 is applied AFTER aggregation (layer 1 aggregates raw x), W2 BEFORE
  (layer 2 aggregates r1@W2), so only one table exchange is needed.
- Edge phase: dst-sorted edges -> 128-edge chunks -> dma_gather (int16
  indices, lo/hi table halves since idx < 32768) -> per-chunk indicator
  ind[p,j] = (j == dloc[p]) * dinv_src[p] -> matmul accumulation.
- Collectives: one AllGather for the layer-2 table, two tiny AllReduces
  for BN stats. Pooling is core-local; output assembled on host.
"""
import sys

sys.path.insert(0, "/opt/trn_rl_repo")

import numpy as np

import concourse.bass as bass
import concourse.bacc as bacc
import concourse.tile as tile
from concourse import mybir
from concourse.bass_utils import run_bass_kernel_spmd

P = 128
NCORES = 8
NG = 64
GPC = NG // NCORES   # graphs per core
GMAX = 8             # max chunks per dma_gather (1024 idx ucode limit)
BAND = 4             # windows per scheduling band
EPS = 1e-5
F32 = mybir.dt.float32
BF16 = mybir.dt.bfloat16
I16 = mybir.dt.int16
I32 = mybir.dt.int32


# ----------------------------------------------------------------------------
# Host preprocessing: padded node space, per-core edge chunks, static schedule
# ----------------------------------------------------------------------------
def _preprocess(x, edge_index, batch):
    x = np.asarray(x, np.float32)
    edge_index = np.asarray(edge_index, np.int64)
    batch = np.asarray(batch, np.int64)
    n_nodes, n_feat = x.shape

    cnt = np.bincount(batch, minlength=NG).astype(np.int64)          # [64]
    gstart = np.zeros(NG + 1, np.int64)
    gstart[1:] = np.cumsum(cnt)
    S = int(np.ceil(max(int(cnt.max()), 1) / 16.0) * 16)             # stride
    SLICE = GPC * S                                                  # per-core cols
    NWIN = SLICE // P
    HALF = 4 * SLICE
    TOT = 8 * SLICE

    # node -> padded id
    i_in_g = np.arange(n_nodes) - gstart[batch]
    pid = (batch // GPC) * SLICE + (batch % GPC) * S + i_in_g        # [n]

    xpad = np.zeros((TOT, n_feat), np.float32)
    xpad[pid] = x

    src = pid[edge_index[0]]
    dst = pid[edge_index[1]]
    src = np.concatenate([src, pid])
    dst = np.concatenate([dst, pid])                                 # + self loops

    deg = np.bincount(dst, minlength=TOT).astype(np.float64)
    dinv = np.where(deg > 0, 1.0 / np.sqrt(deg), 0.0).astype(np.float32)

    # per-core, per-(window, half) edge counts -> static chunk schedule
    core_of = dst // SLICE
    nch = np.zeros((NWIN, 2), np.int64)
    per_core = []
    for c in range(NCORES):
        m = core_of == c
        s_c, d_c = src[m], dst[m] - c * SLICE
        win = d_c // P
        half = (s_c >= HALF).astype(np.int64)
        key = win * 2 + half
        cnt_wh = np.bincount(key, minlength=NWIN * 2).reshape(NWIN, 2)
        nch = np.maximum(nch, (cnt_wh + P - 1) // P)
        per_core.append((s_c, d_c, key))

    # static gather/chunk schedule (band of windows; lo run then hi run)
    gathers = []     # (half, first_chunk, n_chunks)
    chunks = []      # (window, half)
    for b0 in range(0, NWIN, BAND):
        ws = range(b0, min(b0 + BAND, NWIN))
        for h in (0, 1):
            run = [(w, h) for w in ws for _ in range(int(nch[w, h]))]
            i = 0
            while i < len(run):
                n = min(GMAX, len(run) - i)
                gathers.append((h, len(chunks), n))
                chunks.extend(run[i : i + n])
                i += n
    TC = len(chunks)
    first_chunk = {}
    last_chunk = {}
    for ci, (w, _) in enumerate(chunks):
        first_chunk.setdefault(w, ci)
        last_chunk[w] = ci
    empty_wins = [w for w in range(NWIN) if w not in first_chunk]

    slots_of = {}
    for ci, (w, h) in enumerate(chunks):
        slots_of.setdefault((w, h), []).append(ci)

    # per-core metadata
    meta = []
    for c in range(NCORES):
        s_c, d_c, key = per_core[c]
        gidx_c = np.zeros((TC, P), np.int64)
        dloc_c = np.zeros((TC, P), np.float32)
        dsrc_c = np.zeros((TC, P), np.float32)
        order = np.argsort(key, kind="stable")
        so, do = s_c[order], d_c[order]
        ko = key[order]
        bounds = np.searchsorted(ko, np.arange(NWIN * 2 + 1))
        for w in range(NWIN):
            for h in (0, 1):
                k = w * 2 + h
                lo_i, hi_i = int(bounds[k]), int(bounds[k + 1])
                ne = hi_i - lo_i
                if ne == 0:
                    continue
                cix = slots_of[(w, h)]
                flat_g = np.zeros(len(cix) * P, np.int64)
                flat_l = np.zeros(len(cix) * P, np.float32)
                flat_s = np.zeros(len(cix) * P, np.float32)
                ss = so[lo_i:hi_i]
                flat_g[:ne] = ss - (HALF if h else 0)
                flat_l[:ne] = (do[lo_i:hi_i] % P).astype(np.float32)
                flat_s[:ne] = dinv[ss]
                for j, ci in enumerate(cix):
                    gidx_c[ci] = flat_g[j * P : (j + 1) * P]
                    dloc_c[ci] = flat_l[j * P : (j + 1) * P]
                    dsrc_c[ci] = flat_s[j * P : (j + 1) * P]
        # wrap indices per gather: [16, n*8] tiled to 128 partitions
        gidx_w = np.zeros((P, TC * 8), np.int16)
        for h, c0, n in gathers:
            blk = gidx_c[c0 : c0 + n].reshape(n * P)
            w16 = blk.reshape(-1, 16).T.astype(np.int16)
            gidx_w[:, c0 * 8 : (c0 + n) * 8] = np.tile(w16, (8, 1))
        cnt_core = cnt[c * GPC : (c + 1) * GPC].astype(np.float32)
        meta.append(
            dict(
                gidx=np.ascontiguousarray(gidx_w),
                dloc=np.ascontiguousarray(dloc_c.T.astype(np.float32)),
                dsrc=np.ascontiguousarray(dsrc_c.T.astype(np.float32)),
                dinvrep=np.ascontiguousarray(
                    np.tile(dinv[c * SLICE : (c + 1) * SLICE][None, :], (P, 1))
                ),
                padcnt=np.ascontiguousarray(
                    np.tile((S - cnt_core)[None, :], (P, 1)).astype(np.float32)
                ),
                cntg=np.maximum(cnt_core, 1.0).reshape(GPC, 1).astype(np.float32),
            )
        )

    sched = dict(
        S=S, SLICE=SLICE, NWIN=NWIN, HALF=HALF, TOT=TOT, TC=TC,
        gathers=gathers, chunks=chunks, first=first_chunk, last=last_chunk,
        empty_wins=empty_wins, n_nodes=n_nodes, n_feat=n_feat,
    )
    return sched, xpad, meta


# ----------------------------------------------------------------------------
# Device program
# ----------------------------------------------------------------------------
def _edge_phase(nc, sc, sb, ps, gidx_s, dloc_s, dsrc_s, iota_f, tab_lo, tab_hi,
                dinvrep_s, yagg, tag, dt=F32):
    """Gather + indicator + segment matmuls; writes yagg = dinv_dst * agg."""
    psums = {}
    for h, c0, n in sc["gathers"]:
        gat = sb.tile([P, n * P], dt, tag="gat", bufs=2,
                      name=f"gat{tag}_{c0}")
        nc.gpsimd.dma_gather(
            out_ap=gat[:, : n * P].rearrange("p (c d) -> p c d", d=P),
            in_ap=tab_hi if h else tab_lo,
            idxs_ap=gidx_s[:, c0 * 8 : (c0 + n) * 8],
            num_idxs=n * P,
            num_idxs_reg=n * P,
            elem_size=P,
        )
        for j in range(n):
            ci = c0 + j
            w = sc["chunks"][ci][0]
            ind = sb.tile([P, P], dt, tag="ind", bufs=4, name=f"ind{tag}_{ci}")
            nc.vector.tensor_scalar(
                out=ind[:],
                in0=iota_f[:],
                scalar1=dloc_s[:, ci : ci + 1],
                scalar2=dsrc_s[:, ci : ci + 1],
                op0=mybir.AluOpType.is_equal,
                op1=mybir.AluOpType.mult,
            )
            if w not in psums:
                psums[w] = ps.tile([P, P], F32, space="PSUM", tag="acc",
                                   bufs=BAND + 1, name=f"acc{tag}_{w}")
            nc.tensor.matmul(
                out=psums[w][:],
                lhsT=gat[:, j * P : (j + 1) * P],
                rhs=ind[:],
                start=(ci == sc["first"][w]),
                stop=(ci == sc["last"][w]),
            )
            if ci == sc["last"][w]:
                nc.vector.tensor_tensor(
                    out=yagg[:, w * P : (w + 1) * P],
                    in0=psums[w][:],
                    in1=dinvrep_s[:, w * P : (w + 1) * P],
                    op=mybir.AluOpType.mult,
                )
                del psums[w]
    for w in sc["empty_wins"]:
        nc.vector.memset(yagg[:, w * P : (w + 1) * P], 0.0)


def _stats(nc, sb, src_tile, scratch, ncols, tag):
    """[P,2] tile with (sum, sum_sq) over free dim; scratch same size."""
    st = sb.tile([P, 2], F32, name=f"st{tag}")
    nc.vector.reduce_sum(st[:, 0:1], src_tile[:, :ncols], axis=mybir.AxisListType.X)
    nc.vector.tensor_tensor(out=scratch[:, :ncols], in0=src_tile[:, :ncols],
                            in1=src_tile[:, :ncols], op=mybir.AluOpType.mult)
    nc.vector.reduce_sum(st[:, 1:2], scratch[:, :ncols], axis=mybir.AxisListType.X)
    return st


def _bn_affine(nc, sb, dr, st, gamma_s, beta_s, inv_n, tag):
    """AllReduce stats; return (scale, bias) [P,1] tiles for relu(s*y+b)."""
    ar_in = dr.tile([P, 2], F32, name=f"arin{tag}")
    ar_out = dr.tile([P, 2], F32, addr_space="Shared", name=f"arout{tag}")
    nc.gpsimd.dma_start(ar_in[:], st[:])
    nc.gpsimd.collective_compute(
        "AllReduce", mybir.AluOpType.add,
        replica_groups=[list(range(NCORES))],
        ins=[ar_in.opt()], outs=[ar_out.opt()],
    )
    g = sb.tile([P, 2], F32, name=f"g{tag}")
    nc.sync.dma_start(g[:], ar_out[:])
    mom = sb.tile([P, 2], F32, name=f"mom{tag}")
    nc.vector.tensor_scalar_mul(mom[:], g[:], inv_n)
    var = sb.tile([P, 1], F32, name=f"var{tag}")
    nc.vector.tensor_tensor(out=var[:], in0=mom[:, 0:1], in1=mom[:, 0:1],
                            op=mybir.AluOpType.mult)
    nc.vector.tensor_tensor(out=var[:], in0=mom[:, 1:2], in1=var[:],
                            op=mybir.AluOpType.subtract)
    eps_t = sb.tile([P, 1], F32, name=f"eps{tag}")
    nc.vector.memset(eps_t[:], float(EPS))
    sd = sb.tile([P, 1], F32, name=f"sd{tag}")
    nc.scalar.activation(sd[:], var[:], mybir.ActivationFunctionType.Sqrt,
                         bias=eps_t[:, 0:1], scale=1.0)
    inv = sb.tile([P, 1], F32, name=f"inv{tag}")
    nc.vector.reciprocal(inv[:], sd[:])
    scl = sb.tile([P, 1], F32, name=f"scl{tag}")
    nc.vector.tensor_tensor(out=scl[:], in0=gamma_s[:], in1=inv[:],
                            op=mybir.AluOpType.mult)
    bia = sb.tile([P, 1], F32, name=f"bia{tag}")
    nc.vector.tensor_tensor(out=bia[:], in0=mom[:, 0:1], in1=scl[:],
                            op=mybir.AluOpType.mult)
    nc.vector.tensor_tensor(out=bia[:], in0=beta_s[:], in1=bia[:],
                            op=mybir.AluOpType.subtract)
    return scl, bia


def _build_program(sc):
    SLICE, NWIN, HALF, TOT, TC = (sc["SLICE"], sc["NWIN"], sc["HALF"],
                                  sc["TOT"], sc["TC"])
    n_feat = sc["n_feat"]
    nc = bacc.Bacc("TRN2", target_bir_lowering=False, debug=False,
                   num_devices=NCORES)

    xlo_d = nc.dram_tensor("xlo", [HALF, n_feat], F32, kind="ExternalInput")
    xhi_d = nc.dram_tensor("xhi", [HALF, n_feat], F32, kind="ExternalInput")
    gidx_d = nc.dram_tensor("gidx", [P, TC * 8], I16, kind="ExternalInput")
    dloc_d = nc.dram_tensor("dloc", [P, TC], F32, kind="ExternalInput")
    dsrc_d = nc.dram_tensor("dsrc", [P, TC], F32, kind="ExternalInput")
    dinvrep_d = nc.dram_tensor("dinvrep", [P, SLICE], F32, kind="ExternalInput")
    padcnt_d = nc.dram_tensor("padcnt", [P, GPC], F32, kind="ExternalInput")
    cntg_d = nc.dram_tensor("cntg", [GPC, 1], F32, kind="ExternalInput")
    w1_d = nc.dram_tensor("w1", [n_feat, P], F32, kind="ExternalInput")
    w2_d = nc.dram_tensor("w2", [P, P], F32, kind="ExternalInput")
    wfc_d = nc.dram_tensor("wfc", [P, 10], F32, kind="ExternalInput")
    gb_d = nc.dram_tensor("gb", [P, 4], F32, kind="ExternalInput")
    bfc_d = nc.dram_tensor("bfc", [GPC, 10], F32, kind="ExternalInput")
    yout_d = nc.dram_tensor("yout", [GPC, 10], F32, kind="ExternalOutput")

    with tile.TileContext(nc) as tc:
        with tc.tile_pool(name="sbuf", bufs=1) as sb, \
             tc.tile_pool(name="psum", bufs=1, space="PSUM") as ps, \
             tc.tile_pool(name="dram", bufs=1, space="DRAM") as dr:

            iota_i = sb.tile([P, P], I32)
            nc.gpsimd.iota(iota_i[:], pattern=[[1, P]], base=0,
                           channel_multiplier=0)
            iota_f = sb.tile([P, P], F32)
            nc.vector.tensor_copy(iota_f[:], iota_i[:])

            gidx_s = sb.tile([P, TC * 8], I16)
            nc.sync.dma_start(gidx_s[:], gidx_d[:, :])
            dloc_s = sb.tile([P, TC], F32)
            nc.sync.dma_start(dloc_s[:], dloc_d[:, :])
            dsrc_s = sb.tile([P, TC], F32)
            nc.sync.dma_start(dsrc_s[:], dsrc_d[:, :])
            dinvrep_s = sb.tile([P, SLICE], F32)
            nc.sync.dma_start(dinvrep_s[:], dinvrep_d[:, :])
            w1_s = sb.tile([n_feat, P], F32)
            nc.sync.dma_start(w1_s[:], w1_d[:, :])
            w2_s = sb.tile([P, P], F32)
            nc.sync.dma_start(w2_s[:], w2_d[:, :])
            gb_s = sb.tile([P, 4], F32)
            nc.sync.dma_start(gb_s[:], gb_d[:, :])

            # three big shared buffers (see reuse plan in comments below)
            bufA = sb.tile([P, SLICE], F32)   # yagg1 -> sq1 scratch -> y2r
            bufB = sb.tile([P, SLICE], F32)   # y1c  -> yagg2
            bufC = sb.tile([P, SLICE], F32)   # r1   -> sq2 scratch

            # ---------------- layer 1: aggregate raw x ----------------
            _edge_phase(nc, sc, sb, ps, gidx_s, dloc_s, dsrc_s, iota_f,
                        xlo_d[:, :], xhi_d[:, :], dinvrep_s, bufA, tag="a")

            # GEMM1: y1c = W1.T @ yagg  (feat-major)
            for w in range(NWIN):
                pg = ps.tile([P, P], F32, space="PSUM", tag="gemm", bufs=2,
                             name=f"g1_{w}")
                nc.tensor.matmul(out=pg[:], lhsT=w1_s[:],
                                 rhs=bufA[:, w * P : (w + 1) * P],
                                 start=True, stop=True)
                nc.vector.tensor_copy(bufB[:, w * P : (w + 1) * P], pg[:])

            st1 = _stats(nc, sb, bufB, bufA, SLICE, "1")
            s1, b1 = _bn_affine(nc, sb, dr, st1, gb_s[:, 0:1], gb_s[:, 1:2],
                                1.0 / sc["n_nodes"], "1")
            # r1 = relu(s1*y1c + b1)
            nc.scalar.activation(bufC[:], bufB[:],
                                 mybir.ActivationFunctionType.Relu,
                                 bias=b1[:, 0:1], scale=s1[:, 0:1])

            # GEMM2: table2 slice = r1.T @ W2 (node-major rows)
            ag_in = dr.tile([SLICE, P], BF16)
            for w in range(NWIN):
                pg2 = ps.tile([P, P], F32, space="PSUM", tag="gemm", bufs=2,
                              name=f"g2_{w}")
                nc.tensor.matmul(out=pg2[:], lhsT=bufC[:, w * P : (w + 1) * P],
                                 rhs=w2_s[:], start=True, stop=True)
                t2w = sb.tile([P, P], BF16, tag="t2w", bufs=3, name=f"t2w_{w}")
                nc.vector.tensor_copy(t2w[:], pg2[:])
                nc.sync.dma_start(ag_in[w * P : (w + 1) * P, :], t2w[:])

            t2_full = dr.tile([TOT, P], BF16, addr_space="Shared")
            nc.gpsimd.collective_compute(
                "AllGather", mybir.AluOpType.bypass,
                replica_groups=[list(range(NCORES))],
                ins=[ag_in.opt()], outs=[t2_full.opt()],
            )

            # ---------------- layer 2: aggregate table2 ----------------
            _edge_phase(nc, sc, sb, ps, gidx_s, dloc_s, dsrc_s, iota_f,
                        t2_full[0:HALF, :], t2_full[HALF:TOT, :],
                        dinvrep_s, bufB, tag="b", dt=BF16)

            st2 = _stats(nc, sb, bufB, bufC, SLICE, "2")
            s2, b2 = _bn_affine(nc, sb, dr, st2, gb_s[:, 2:3], gb_s[:, 3:4],
                                1.0 / sc["n_nodes"], "2")
            # y2r = relu(s2*y2 + b2)
            nc.scalar.activation(bufA[:], bufB[:],
                                 mybir.ActivationFunctionType.Relu,
                                 bias=b2[:, 0:1], scale=s2[:, 0:1])

            # pool: per-graph sums minus pad correction
            pooled = sb.tile([P, GPC], F32)
            for j in range(GPC):
                nc.vector.reduce_sum(pooled[:, j : j + 1],
                                     bufA[:, j * sc["S"] : (j + 1) * sc["S"]],
                                     axis=mybir.AxisListType.X)
            rb = sb.tile([P, 1], F32)
            nc.scalar.activation(rb[:], b2[:, 0:1],
                                 mybir.ActivationFunctionType.Relu)
            padcnt_s = sb.tile([P, GPC], F32)
            nc.sync.dma_start(padcnt_s[:], padcnt_d[:, :])
            corr = sb.tile([P, GPC], F32)
            nc.vector.tensor_scalar_mul(corr[:], padcnt_s[:], rb[:, 0:1])
            nc.vector.tensor_tensor(out=pooled[:], in0=pooled[:], in1=corr[:],
                                    op=mybir.AluOpType.subtract)

            # FC: out[g, cls] = pooled.T @ Wfc / cnt + bfc
            wfc_s = sb.tile([P, 10], F32)
            nc.sync.dma_start(wfc_s[:], wfc_d[:, :])
            cnt_s = sb.tile([GPC, 1], F32)
            nc.sync.dma_start(cnt_s[:], cntg_d[:, :])
            cinv = sb.tile([GPC, 1], F32)
            nc.vector.reciprocal(cinv[:], cnt_s[:])
            bfc_s = sb.tile([GPC, 10], F32)
            nc.sync.dma_start(bfc_s[:], bfc_d[:, :])
            pfc = ps.tile([GPC, 10], F32, space="PSUM")
            nc.tensor.matmul(out=pfc[:], lhsT=pooled[:], rhs=wfc_s[:],
                             start=True, stop=True)
            yo = sb.tile([GPC, 10], F32)
            nc.vector.tensor_scalar_mul(yo[:], pfc[:], cinv[:, 0:1])
            nc.vector.tensor_tensor(out=yo[:], in0=yo[:], in1=bfc_s[:],
                                    op=mybir.AluOpType.add)
            nc.sync.dma_start(yout_d[:, :], yo[:])

    nc.compile()
    return nc


# ----------------------------------------------------------------------------
# Entry point
# ----------------------------------------------------------------------------
def _make_in_maps(sc, xpad, meta, inputs):
    HALF = sc["HALF"]
    gb = np.stack([np.asarray(inputs["gamma1"], np.float32),
                   np.asarray(inputs["beta1"], np.float32),
                   np.asarray(inputs["gamma2"], np.float32),
                   np.asarray(inputs["beta2"], np.float32)], axis=1)  # [128, 4]
    common = dict(
        xlo=np.ascontiguousarray(xpad[:HALF]),
        xhi=np.ascontiguousarray(xpad[HALF:]),
        w1=np.ascontiguousarray(np.asarray(inputs["W1"], np.float32)),
        w2=np.ascontiguousarray(np.asarray(inputs["W2"], np.float32)),
        wfc=np.ascontiguousarray(np.asarray(inputs["Wfc"], np.float32)),
        gb=np.ascontiguousarray(gb),
    )
    bfc_rep = np.ascontiguousarray(
        np.tile(np.asarray(inputs["bfc"], np.float32)[None, :], (GPC, 1)))
    in_maps = []
    for c in range(NCORES):
        m = meta[c]
        in_maps.append(dict(common, gidx=m["gidx"], dloc=m["dloc"],
                            dsrc=m["dsrc"], dinvrep=m["dinvrep"],
                            padcnt=m["padcnt"], cntg=m["cntg"], bfc=bfc_rep))
    return in_maps


def kernel(x, edge_index, batch, W1, b1, gamma1, beta1, W2, b2, gamma2, beta2,
           Wfc, bfc, _trace=False):
    sc, xpad, meta = _preprocess(x, edge_index, batch)
    nc = _build_program(sc)
    in_maps = _make_in_maps(sc, xpad, meta, dict(
        gamma1=gamma1, beta1=beta1, gamma2=gamma2, beta2=beta2,
        W1=W1, W2=W2, Wfc=Wfc, bfc=bfc))

    res = None
    last_err = None
    for _attempt in range(3):
        try:
            res = run_bass_kernel_spmd(nc, in_maps, core_ids=list(range(NCORES)),
                                       trace=_trace)
            break
        except Exception as e:  # transient NRT/axon failures; retry
            last_err = e
    if res is None:
        raise last_err
    out = np.concatenate([res.results[c]["yout"] for c in range(NCORES)], axis=0)
    if _trace:
        return out.astype(np.float32), res
    return out.astype(np.float32)


# revision 9
# speedup vs baseline: 1.5389x; 1.3560x over previous
"""GCN (2x GCNConv + BN + ReLU, mean-pool, FC) on 8 TRN2 NeuronCores.

Strategy (1D graph partition by destination node):
- Nodes are permuted into a graph-strided padded space: core c owns graphs
  [8c, 8c+8), each graph padded to a fixed stride S -> per-core slice of
  SLICE = 8*S columns. This makes BN/pool/slicing static across the SPMD
  program (one program, 8 data).
- conv = D^-1/2 (A+I) D^-1/2 (h @ W) is refactored: gather raw table rows
  h[src], scale by dinv[src] inside the indicator matrix, segment-sum via
  PE matmuls into [feat, dst] PSUM windows, scale by dinv[dst] afterwards.
  W1 is applied AFTER aggregation (layer 1 aggregates raw x), W2 BEFORE
  (layer 2 aggregates r1@W2), so only one table exchange is needed.
- Edge phase: dst-sorted edges -> 128-edge chunks -> dma_gather (int16
  indices, lo/hi table halves since idx < 32768) -> per-chunk indicator
  ind[p,j] = (j == dloc[p]) * dinv_src[p] -> matmul accumulation.
- Collectives: one AllGather for the layer-2 table, two tiny AllReduces
  for BN stats. Pooling is core-local; output assembled on host.
"""
import sys

sys.path.insert(0, "/opt/trn_rl_repo")

import numpy as np

import concourse.bass as bass
import concourse.bacc as bacc
import concourse.tile as tile
from concourse import mybir
from concourse.bass_utils import run_bass_kernel_spmd

P = 128
NCORES = 8
NG = 64
GPC = NG // NCORES   # graphs per core
GMAX = 8             # max chunks per dma_gather (1024 idx ucode limit)
BAND = 4             # windows per scheduling band
EPS = 1e-5
F32 = mybir.dt.float32
BF16 = mybir.dt.bfloat16
I16 = mybir.dt.int16
I32 = mybir.dt.int32


# ----------------------------------------------------------------------------
# Host preprocessing: padded node space, per-core edge chunks, static schedule
# ----------------------------------------------------------------------------
def _preprocess(x, edge_index, batch):
    x = np.asarray(x, np.float32)
    edge_index = np.asarray(edge_index, np.int64)
    batch = np.asarray(batch, np.int64)
    n_nodes, n_feat = x.shape

    cnt = np.bincount(batch, minlength=NG).astype(np.int64)          # [64]
    gstart = np.zeros(NG + 1, np.int64)
    gstart[1:] = np.cumsum(cnt)
    S = int(np.ceil(max(int(cnt.max()), 1) / 16.0) * 16)             # stride
    SLICE = GPC * S                                                  # per-core cols
    NWIN = SLICE // P
    HALF = 4 * SLICE
    TOT = 8 * SLICE

    # node -> padded id
    i_in_g = np.arange(n_nodes) - gstart[batch]
    pid = (batch // GPC) * SLICE + (batch % GPC) * S + i_in_g        # [n]

    xpad = np.zeros((TOT, n_feat), np.float32)
    xpad[pid] = x

    src = pid[edge_index[0]]
    dst = pid[edge_index[1]]
    src = np.concatenate([src, pid])
    dst = np.concatenate([dst, pid])                                 # + self loops

    deg = np.bincount(dst, minlength=TOT).astype(np.float64)
    dinv = np.where(deg > 0, 1.0 / np.sqrt(deg), 0.0).astype(np.float32)

    # per-core, per-(window, half) edge counts -> static chunk schedule
    core_of = dst // SLICE
    nch = np.zeros((NWIN, 2), np.int64)
    per_core = []
    for c in range(NCORES):
        m = core_of == c
        s_c, d_c = src[m], dst[m] - c * SLICE
        win = d_c // P
        half = (s_c >= HALF).astype(np.int64)
        key = win * 2 + half
        cnt_wh = np.bincount(key, minlength=NWIN * 2).reshape(NWIN, 2)
        nch = np.maximum(nch, (cnt_wh + P - 1) // P)
        per_core.append((s_c, d_c, key))

    # static gather/chunk schedule (band of windows; lo run then hi run)
    gathers = []     # (half, first_chunk, n_chunks)
    chunks = []      # (window, half)
    for b0 in range(0, NWIN, BAND):
        ws = range(b0, min(b0 + BAND, NWIN))
        for h in (0, 1):
            run = [(w, h) for w in ws for _ in range(int(nch[w, h]))]
            i = 0
            while i < len(run):
                n = min(GMAX, len(run) - i)
                gathers.append((h, len(chunks), n))
                chunks.extend(run[i : i + n])
                i += n
    TC = len(chunks)
    first_chunk = {}
    last_chunk = {}
    for ci, (w, _) in enumerate(chunks):
        first_chunk.setdefault(w, ci)
        last_chunk[w] = ci
    empty_wins = [w for w in range(NWIN) if w not in first_chunk]

    slots_of = {}
    for ci, (w, h) in enumerate(chunks):
        slots_of.setdefault((w, h), []).append(ci)

    # per-core metadata
    meta = []
    for c in range(NCORES):
        s_c, d_c, key = per_core[c]
        gidx_c = np.zeros((TC, P), np.int64)
        dloc_c = np.zeros((TC, P), np.float32)
        dsrc_c = np.zeros((TC, P), np.float32)
        order = np.argsort(key, kind="stable")
        so, do = s_c[order], d_c[order]
        ko = key[order]
        bounds = np.searchsorted(ko, np.arange(NWIN * 2 + 1))
        for w in range(NWIN):
            for h in (0, 1):
                k = w * 2 + h
                lo_i, hi_i = int(bounds[k]), int(bounds[k + 1])
                ne = hi_i - lo_i
                if ne == 0:
                    continue
                cix = slots_of[(w, h)]
                flat_g = np.zeros(len(cix) * P, np.int64)
                flat_l = np.zeros(len(cix) * P, np.float32)
                flat_s = np.zeros(len(cix) * P, np.float32)
                ss = so[lo_i:hi_i]
                flat_g[:ne] = ss - (HALF if h else 0)
                flat_l[:ne] = (do[lo_i:hi_i] % P).astype(np.float32)
                flat_s[:ne] = dinv[ss]
                for j, ci in enumerate(cix):
                    gidx_c[ci] = flat_g[j * P : (j + 1) * P]
                    dloc_c[ci] = flat_l[j * P : (j + 1) * P]
                    dsrc_c[ci] = flat_s[j * P : (j + 1) * P]
        # wrap indices per gather: [16, n*8] tiled to 128 partitions
        gidx_w = np.zeros((P, TC * 8), np.int16)
        for h, c0, n in gathers:
            blk = gidx_c[c0 : c0 + n].reshape(n * P)
            w16 = blk.reshape(-1, 16).T.astype(np.int16)
            gidx_w[:, c0 * 8 : (c0 + n) * 8] = np.tile(w16, (8, 1))
        cnt_core = cnt[c * GPC : (c + 1) * GPC].astype(np.float32)
        meta.append(
            dict(
                gidx=np.ascontiguousarray(gidx_w),
                dloc=np.ascontiguousarray(dloc_c.T.astype(np.float32)),
                dsrc=np.ascontiguousarray(dsrc_c.T.astype(np.float32)),
                dinvrep=np.ascontiguousarray(
                    np.tile(dinv[c * SLICE : (c + 1) * SLICE][None, :], (P, 1))
                ),
                padcnt=np.ascontiguousarray(
                    np.tile((S - cnt_core)[None, :], (P, 1)).astype(np.float32)
                ),
                cntg=np.maximum(cnt_core, 1.0).reshape(GPC, 1).astype(np.float32),
            )
        )

    sched = dict(
        S=S, SLICE=SLICE, NWIN=NWIN, HALF=HALF, TOT=TOT, TC=TC,
        gathers=gathers, chunks=chunks, first=first_chunk, last=last_chunk,
        empty_wins=empty_wins, n_nodes=n_nodes, n_feat=n_feat,
    )
    return sched, xpad, meta


# ----------------------------------------------------------------------------
# Device program
# ----------------------------------------------------------------------------
def _edge_phase(nc, sc, sb, ps, gidx_s, dloc_s, dsrc_s, iota_f, tab_lo, tab_hi,
                dinvrep_s, yagg, tag, dt=F32):
    """Gather + indicator + segment matmuls; writes yagg = dinv_dst * agg."""
    psums = {}
    for h, c0, n in sc["gathers"]:
        gat = sb.tile([P, n * P], dt, tag="gat", bufs=4,
                      name=f"gat{tag}_{c0}")
        nc.gpsimd.dma_gather(
            out_ap=gat[:, : n * P].rearrange("p (c d) -> p c d", d=P),
            in_ap=tab_hi if h else tab_lo,
            idxs_ap=gidx_s[:, c0 * 8 : (c0 + n) * 8],
            num_idxs=n * P,
            num_idxs_reg=n * P,
            elem_size=P,
        )
        for j in range(n):
            ci = c0 + j
            w = sc["chunks"][ci][0]
            ind = sb.tile([P, P], dt, tag="ind", bufs=8, name=f"ind{tag}_{ci}")
            nc.vector.tensor_scalar(
                out=ind[:],
                in0=iota_f[:],
                scalar1=dloc_s[:, ci : ci + 1],
                scalar2=dsrc_s[:, ci : ci + 1],
                op0=mybir.AluOpType.is_equal,
                op1=mybir.AluOpType.mult,
            )
            if w not in psums:
                psums[w] = ps.tile([P, P], F32, space="PSUM", tag="acc",
                                   bufs=BAND + 1, name=f"acc{tag}_{w}")
            nc.tensor.matmul(
                out=psums[w][:],
                lhsT=gat[:, j * P : (j + 1) * P],
                rhs=ind[:],
                start=(ci == sc["first"][w]),
                stop=(ci == sc["last"][w]),
            )
            if ci == sc["last"][w]:
                nc.vector.tensor_tensor(
                    out=yagg[:, w * P : (w + 1) * P],
                    in0=psums[w][:],
                    in1=dinvrep_s[:, w * P : (w + 1) * P],
                    op=mybir.AluOpType.mult,
                )
                del psums[w]
    for w in sc["empty_wins"]:
        nc.vector.memset(yagg[:, w * P : (w + 1) * P], 0.0)


def _stats(nc, sb, src_tile, scratch, ncols, tag):
    """[P,2] tile with (sum, sum_sq) over free dim; scratch same size."""
    st = sb.tile([P, 2], F32, name=f"st{tag}")
    nc.vector.reduce_sum(st[:, 0:1], src_tile[:, :ncols], axis=mybir.AxisListType.X)
    nc.vector.tensor_tensor(out=scratch[:, :ncols], in0=src_tile[:, :ncols],
                            in1=src_tile[:, :ncols], op=mybir.AluOpType.mult)
    nc.vector.reduce_sum(st[:, 1:2], scratch[:, :ncols], axis=mybir.AxisListType.X)
    return st


def _bn_affine(nc, sb, dr, st, gamma_s, beta_s, inv_n, tag):
    """AllReduce stats; return (scale, bias) [P,1] tiles for relu(s*y+b)."""
    ar_in = dr.tile([P, 2], F32, name=f"arin{tag}")
    ar_out = dr.tile([P, 2], F32, addr_space="Shared", name=f"arout{tag}")
    nc.gpsimd.dma_start(ar_in[:], st[:])
    nc.gpsimd.collective_compute(
        "AllReduce", mybir.AluOpType.add,
        replica_groups=[list(range(NCORES))],
        ins=[ar_in.opt()], outs=[ar_out.opt()],
    )
    g = sb.tile([P, 2], F32, name=f"g{tag}")
    nc.sync.dma_start(g[:], ar_out[:])
    mom = sb.tile([P, 2], F32, name=f"mom{tag}")
    nc.vector.tensor_scalar_mul(mom[:], g[:], inv_n)
    var = sb.tile([P, 1], F32, name=f"var{tag}")
    nc.vector.tensor_tensor(out=var[:], in0=mom[:, 0:1], in1=mom[:, 0:1],
                            op=mybir.AluOpType.mult)
    nc.vector.tensor_tensor(out=var[:], in0=mom[:, 1:2], in1=var[:],
                            op=mybir.AluOpType.subtract)
    eps_t = sb.tile([P, 1], F32, name=f"eps{tag}")
    nc.vector.memset(eps_t[:], float(EPS))
    sd = sb.tile([P, 1], F32, name=f"sd{tag}")
    nc.scalar.activation(sd[:], var[:], mybir.ActivationFunctionType.Sqrt,
                         bias=eps_t[:, 0:1], scale=1.0)
    inv = sb.tile([P, 1], F32, name=f"inv{tag}")
    nc.vector.reciprocal(inv[:], sd[:])
    scl = sb.tile([P, 1], F32, name=f"scl{tag}")
    nc.vector.tensor_tensor(out=scl[:], in0=gamma_s[:], in1=inv[:],
                            op=mybir.AluOpType.mult)
    bia = sb.tile([P, 1], F32, name=f"bia{tag}")
    nc.vector.tensor_tensor(out=bia[:], in0=mom[:, 0:1], in1=scl[:],
                            op=mybir.AluOpType.mult)
    nc.vector.tensor_tensor(out=bia[:], in0=beta_s[:], in1=bia[:],
                            op=mybir.AluOpType.subtract)
    return scl, bia


def _build_program(sc):
    SLICE, NWIN, HALF, TOT, TC = (sc["SLICE"], sc["NWIN"], sc["HALF"],
                                  sc["TOT"], sc["TC"])
    n_feat = sc["n_feat"]
    nc = bacc.Bacc("TRN2", target_bir_lowering=False, debug=False,
                   num_devices=NCORES)

    xlo_d = nc.dram_tensor("xlo", [HALF, n_feat], F32, kind="ExternalInput")
    xhi_d = nc.dram_tensor("xhi", [HALF, n_feat], F32, kind="ExternalInput")
    gidx_d = nc.dram_tensor("gidx", [P, TC * 8], I16, kind="ExternalInput")
    dloc_d = nc.dram_tensor("dloc", [P, TC], F32, kind="ExternalInput")
    dsrc_d = nc.dram_tensor("dsrc", [P, TC], F32, kind="ExternalInput")
    dinvrep_d = nc.dram_tensor("dinvrep", [P, SLICE], F32, kind="ExternalInput")
    padcnt_d = nc.dram_tensor("padcnt", [P, GPC], F32, kind="ExternalInput")
    cntg_d = nc.dram_tensor("cntg", [GPC, 1], F32, kind="ExternalInput")
    w1_d = nc.dram_tensor("w1", [n_feat, P], F32, kind="ExternalInput")
    w2_d = nc.dram_tensor("w2", [P, P], F32, kind="ExternalInput")
    wfc_d = nc.dram_tensor("wfc", [P, 10], F32, kind="ExternalInput")
    gb_d = nc.dram_tensor("gb", [P, 4], F32, kind="ExternalInput")
    bfc_d = nc.dram_tensor("bfc", [GPC, 10], F32, kind="ExternalInput")
    yout_d = nc.dram_tensor("yout", [GPC, 10], F32, kind="ExternalOutput")

    with tile.TileContext(nc) as tc:
        with tc.tile_pool(name="sbuf", bufs=1) as sb, \
             tc.tile_pool(name="psum", bufs=1, space="PSUM") as ps, \
             tc.tile_pool(name="dram", bufs=1, space="DRAM") as dr:

            iota_i = sb.tile([P, P], I32)
            nc.gpsimd.iota(iota_i[:], pattern=[[1, P]], base=0,
                           channel_multiplier=0)
            iota_f = sb.tile([P, P], F32)
            nc.vector.tensor_copy(iota_f[:], iota_i[:])

            gidx_s = sb.tile([P, TC * 8], I16)
            nc.sync.dma_start(gidx_s[:], gidx_d[:, :])
            dloc_s = sb.tile([P, TC], F32)
            nc.sync.dma_start(dloc_s[:], dloc_d[:, :])
            dsrc_s = sb.tile([P, TC], F32)
            nc.sync.dma_start(dsrc_s[:], dsrc_d[:, :])
            dinvrep_s = sb.tile([P, SLICE], F32)
            nc.sync.dma_start(dinvrep_s[:], dinvrep_d[:, :])
            w1_s = sb.tile([n_feat, P], F32)
            nc.sync.dma_start(w1_s[:], w1_d[:, :])
            w2_s = sb.tile([P, P], F32)
            nc.sync.dma_start(w2_s[:], w2_d[:, :])
            gb_s = sb.tile([P, 4], F32)
            nc.sync.dma_start(gb_s[:], gb_d[:, :])

            # three big shared buffers (see reuse plan in comments below)
            bufA = sb.tile([P, SLICE], F32)   # yagg1 -> sq1 scratch -> y2r
            bufB = sb.tile([P, SLICE], F32)   # y1c  -> yagg2
            bufC = sb.tile([P, SLICE], F32)   # r1   -> sq2 scratch

            # ---------------- layer 1: aggregate raw x ----------------
            _edge_phase(nc, sc, sb, ps, gidx_s, dloc_s, dsrc_s, iota_f,
                        xlo_d[:, :], xhi_d[:, :], dinvrep_s, bufA, tag="a")

            # GEMM1: y1c = W1.T @ yagg  (feat-major)
            for w in range(NWIN):
                pg = ps.tile([P, P], F32, space="PSUM", tag="gemm", bufs=2,
                             name=f"g1_{w}")
                nc.tensor.matmul(out=pg[:], lhsT=w1_s[:],
                                 rhs=bufA[:, w * P : (w + 1) * P],
                                 start=True, stop=True)
                nc.vector.tensor_copy(bufB[:, w * P : (w + 1) * P], pg[:])

            st1 = _stats(nc, sb, bufB, bufA, SLICE, "1")
            s1, b1 = _bn_affine(nc, sb, dr, st1, gb_s[:, 0:1], gb_s[:, 1:2],
                                1.0 / sc["n_nodes"], "1")
            # r1 = relu(s1*y1c + b1)
            nc.scalar.activation(bufC[:], bufB[:],
                                 mybir.ActivationFunctionType.Relu,
                                 bias=b1[:, 0:1], scale=s1[:, 0:1])

            # GEMM2: table2 slice = r1.T @ W2 (node-major rows)
            ag_in = dr.tile([SLICE, P], BF16)
            for w in range(NWIN):
                pg2 = ps.tile([P, P], F32, space="PSUM", tag="gemm", bufs=2,
                              name=f"g2_{w}")
                nc.tensor.matmul(out=pg2[:], lhsT=bufC[:, w * P : (w + 1) * P],
                                 rhs=w2_s[:], start=True, stop=True)
                t2w = sb.tile([P, P], BF16, tag="t2w", bufs=3, name=f"t2w_{w}")
                nc.vector.tensor_copy(t2w[:], pg2[:])
                nc.sync.dma_start(ag_in[w * P : (w + 1) * P, :], t2w[:])

            t2_full = dr.tile([TOT, P], BF16, addr_space="Shared")
            nc.gpsimd.collective_compute(
                "AllGather", mybir.AluOpType.bypass,
                replica_groups=[list(range(NCORES))],
                ins=[ag_in.opt()], outs=[t2_full.opt()],
            )

            # ---------------- layer 2: aggregate table2 ----------------
            _edge_phase(nc, sc, sb, ps, gidx_s, dloc_s, dsrc_s, iota_f,
                        t2_full[0:HALF, :], t2_full[HALF:TOT, :],
                        dinvrep_s, bufB, tag="b", dt=BF16)

            st2 = _stats(nc, sb, bufB, bufC, SLICE, "2")
            s2, b2 = _bn_affine(nc, sb, dr, st2, gb_s[:, 2:3], gb_s[:, 3:4],
                                1.0 / sc["n_nodes"], "2")
            # y2r = relu(s2*y2 + b2)
            nc.scalar.activation(bufA[:], bufB[:],
                                 mybir.ActivationFunctionType.Relu,
                                 bias=b2[:, 0:1], scale=s2[:, 0:1])

            # pool: per-graph sums minus pad correction
            pooled = sb.tile([P, GPC], F32)
            for j in range(GPC):
                nc.vector.reduce_sum(pooled[:, j : j + 1],
                                     bufA[:, j * sc["S"] : (j + 1) * sc["S"]],
                                     axis=mybir.AxisListType.X)
            rb = sb.tile([P, 1], F32)
            nc.scalar.activation(rb[:], b2[:, 0:1],
                                 mybir.ActivationFunctionType.Relu)
            padcnt_s = sb.tile([P, GPC], F32)
            nc.sync.dma_start(padcnt_s[:], padcnt_d[:, :])
            corr = sb.tile([P, GPC], F32)
            nc.vector.tensor_scalar_mul(corr[:], padcnt_s[:], rb[:, 0:1])
            nc.vector.tensor_tensor(out=pooled[:], in0=pooled[:], in1=corr[:],
                                    op=mybir.AluOpType.subtract)

            # FC: out[g, cls] = pooled.T @ Wfc / cnt + bfc
            wfc_s = sb.tile([P, 10], F32)
            nc.sync.dma_start(wfc_s[:], wfc_d[:, :])
            cnt_s = sb.tile([GPC, 1], F32)
            nc.sync.dma_start(cnt_s[:], cntg_d[:, :])
            cinv = sb.tile([GPC, 1], F32)
            nc.vector.reciprocal(cinv[:], cnt_s[:])
            bfc_s = sb.tile([GPC, 10], F32)
            nc.sync.dma_start(bfc_s[:], bfc_d[:, :])
            pfc = ps.tile([GPC, 10], F32, space="PSUM")
            nc.tensor.matmul(out=pfc[:], lhsT=pooled[:], rhs=wfc_s[:],
                             start=True, stop=True)
            yo = sb.tile([GPC, 10], F32)
            nc.vector.tensor_scalar_mul(yo[:], pfc[:], cinv[:, 0:1])
            nc.vector.tensor_tensor(out=yo[:], in0=yo[:], in1=bfc_s[:],
                                    op=mybir.AluOpType.add)
            nc.sync.dma_start(yout_d[:, :], yo[:])

    nc.compile()
    return nc


# ----------------------------------------------------------------------------
# Entry point
# ----------------------------------------------------------------------------
def _make_in_maps(sc, xpad, meta, inputs):
    HALF = sc["HALF"]
    gb = np.stack([np.asarray(inputs["gamma1"], np.float32),
                   np.asarray(inputs["beta1"], np.float32),
                   np.asarray(inputs["gamma2"], np.float32),
                   np.asarray(inputs["beta2"], np.float32)], axis=1)  # [128, 4]
    common = dict(
        xlo=np.ascontiguousarray(xpad[:HALF]),
        xhi=np.ascontiguousarray(xpad[HALF:]),
        w1=np.ascontiguousarray(np.asarray(inputs["W1"], np.float32)),
        w2=np.ascontiguousarray(np.asarray(inputs["W2"], np.float32)),
        wfc=np.ascontiguousarray(np.asarray(inputs["Wfc"], np.float32)),
        gb=np.ascontiguousarray(gb),
    )
    bfc_rep = np.ascontiguousarray(
        np.tile(np.asarray(inputs["bfc"], np.float32)[None, :], (GPC, 1)))
    in_maps = []
    for c in range(NCORES):
        m = meta[c]
        in_maps.append(dict(common, gidx=m["gidx"], dloc=m["dloc"],
                            dsrc=m["dsrc"], dinvrep=m["dinvrep"],
                            padcnt=m["padcnt"], cntg=m["cntg"], bfc=bfc_rep))
    return in_maps


def kernel(x, edge_index, batch, W1, b1, gamma1, beta1, W2, b2, gamma2, beta2,
           Wfc, bfc, _trace=False):
    sc, xpad, meta = _preprocess(x, edge_index, batch)
    nc = _build_program(sc)
    in_maps = _make_in_maps(sc, xpad, meta, dict(
        gamma1=gamma1, beta1=beta1, gamma2=gamma2, beta2=beta2,
        W1=W1, W2=W2, Wfc=Wfc, bfc=bfc))

    res = None
    last_err = None
    for _attempt in range(3):
        try:
            res = run_bass_kernel_spmd(nc, in_maps, core_ids=list(range(NCORES)),
                                       trace=_trace)
            break
        except Exception as e:  # transient NRT/axon failures; retry
            last_err = e
    if res is None:
        raise last_err
    out = np.concatenate([res.results[c]["yout"] for c in range(NCORES)], axis=0)
    if _trace:
        return out.astype(np.float32), res
    return out.astype(np.float32)
